# revision 2
# baseline (speedup 1.0000x reference)
"""Trainium2 Bass kernel for nn_Attention_4243427688485.

Computation (per batch b):
    a   = z_b @ M @ e_b^T            [N, ME]
    A   = softmax(sigmoid(a), dim=N) (softmax over the query axis N)
    eo  = A @ e_b                    [N, D]
Returns (eo, A) stacked over the batch.

Sharding: data-parallel over batch B=8 across the 8 NeuronCores (one batch
per core, M replicated).  No collectives.  Host uploads fp16 shards
(z/e pre-transposed); outputs come back fp16 and transposed where noted.

Per-core device program:
  - mm1 (fp16): zMT[e',n] = sum_d M[d,e'] z[n,d]
  - mm2 (fp16): aT[m,n]   = sum_e' e[m,e'] zM[n,e']; ScalarE evicts via
    tanh(a/2); softmax over n: t = exp(0.5u+0.5), accum_out row-sum S,
    DVE reciprocal r=1/S, aT16 = t*r (fp16) = the A output (transposed).
  - mm3 (fp8 DoubleRow, ~1.5-2x TensorE rate): exploits the sigmoid
    saturation structure.  t = exp(sigmoid(a)) clusters at exactly {1, e}
    (98.7% of entries saturate), so with the affine split
        t = c + beta*h,  c=(1+e)/2, beta=(e-1)/2,  h in {-1,+1} (mostly)
    h is EXACTLY representable in fp8e4 at the clusters.  Then
        eo[n,d] = c * colsum[d] + beta * sum_m h[m,n] * (e[m,d]/S[m])
    The beta-term runs as fp8e4 DoubleRow matmuls in the eoT orientation:
        eoT = (etil8)^T-style matmul: lhsT = etil8[m,d] = fp8(e*r*2048),
        rhs = h8[m,n], psum accumulates m in 4 double-row (256-wide) steps.
    The c*colsum term is rank-1 in n: the device outputs S ([128,8] fp32,
    4KB) and the host folds  eo += c * ((1/S) @ e)  exactly in fp32 during
    the gather (1M MACs/batch on host, negligible).
    fp8 quantization error on etil dominates: measured rel_err(eo) ~1.1e-2
    vs the 2e-2 gate (A output unchanged at ~2.8e-3).
"""

import numpy as np

import concourse.bass as bass
import concourse.mybir as mybir
import concourse.tile as tile
from concourse import bacc
from concourse.bass_utils import run_bass_kernel_spmd

AF = mybir.ActivationFunctionType
ALU = mybir.AluOpType
DR = mybir.MatmulPerfMode.DoubleRow
F32 = mybir.dt.float32
FP16 = mybir.dt.float16
FP8 = mybir.dt.float8e4

P = 128
NT = 8
SZ = 1024
NC = 8

C_AFF = (1.0 + float(np.e)) / 2.0     # 1.8591409142295225
B_AFF = (float(np.e) - 1.0) / 2.0     # 0.8591409142295225
KQ = 2048.0                           # etil prescale so fp8e4 sees ~unit values


def _build_nc(unroll: int = 1, tiny_io: bool = False) -> bass.Bass:
    nc = bacc.Bacc()

    if tiny_io:
        nc.declare_dram_parameter("tin", [1, 1], F32, isOutput=False)
        dout = nc.declare_dram_parameter("tout", [1, 1], F32, isOutput=True)
        zt_d = nc.dram_tensor("zti", [SZ, SZ], FP16)
        e_d = nc.dram_tensor("ei", [SZ, SZ], FP16)
        et_d = nc.dram_tensor("eti", [SZ, SZ], FP16)
        m_d = nc.dram_tensor("Mi", [SZ, SZ], FP16)
        eo_d = nc.dram_tensor("eoi", [SZ, SZ], FP16)
        a_d = nc.dram_tensor("Ai", [SZ, SZ], FP16)
        s_d = nc.dram_tensor("Si", [P, NT], F32)
    else:
        zt_d = nc.declare_dram_parameter("zT", [SZ, SZ], FP16, isOutput=False)
        e_d = nc.declare_dram_parameter("e", [SZ, SZ], FP16, isOutput=False)
        et_d = nc.declare_dram_parameter("eT", [SZ, SZ], FP16, isOutput=False)
        m_d = nc.declare_dram_parameter("M", [SZ, SZ], FP16, isOutput=False)
        eo_d = nc.declare_dram_parameter("eoT", [SZ, SZ], FP16, isOutput=True)
        a_d = nc.declare_dram_parameter("A", [SZ, SZ], FP16, isOutput=True)
        s_d = nc.declare_dram_parameter("S", [P, NT], F32, isOutput=True)

    ztr = zt_d.rearrange("(j p) d -> j p d", p=P)
    er = e_d.rearrange("(j p) d -> j p d", p=P)
    etr = et_d.rearrange("(j p) d -> j p d", p=P)
    mr = m_d.rearrange("(j p) d -> j p d", p=P)
    eor = eo_d.rearrange("(j p) d -> j p d", p=P)
    ar = a_d.rearrange("(j p) d -> j p d", p=P)

    with tile.TileContext(nc) as tc:
        with (
            tc.tile_pool(name="big", bufs=1) as big,
            tc.tile_pool(name="consts", bufs=1) as consts,
            tc.tile_pool(name="tpool", bufs=4) as tpool,
            tc.tile_pool(name="stage", bufs=8) as stage,
            tc.tile_pool(name="psum_mm", bufs=1, space="PSUM") as pmm,
        ):
            halfb = consts.tile([P, 1], F32)
            nc.any.memset(halfb, 0.5)
            zerob = consts.tile([P, 1], F32)
            nc.any.memset(zerob, 0.0)
            S = consts.tile([P, NT], F32)
            r = consts.tile([P, NT], F32)

            m16 = big.tile([P, NT, SZ], FP16)    # m16[p, jd, e'] = M[jd*128+p, e']
            zT16 = big.tile([P, NT, SZ], FP16)   # zT16[p, jd, n] = z[n, jd*128+p]
            e16 = big.tile([P, NT, SZ], FP16)    # e16[p, jm, d]  = e[jm*128+p, d]
            eT16 = big.tile([P, NT, SZ], FP16)   # eT16[p, je, m] = e[m, je*128+p]
            zMT = big.tile([P, NT, SZ], FP16)    # zMT[p, je, n]  = (z@M)[n, je*128+p]
            u16 = big.tile([P, NT, SZ], FP16)    # u[p, jm, n]    = tanh(a[n, jm*128+p]/2)
            aT16 = big.tile([P, NT, SZ], FP16)   # aT16[p, jm, n] = A[n, jm*128+p]
            h8 = big.tile([P, NT, SZ], FP8)      # h8[p, jm, n]   = (t - c)/beta
            etil8 = big.tile([P, NT, SZ], FP8)   # etil8[p, jm, d]= e[m,d]*r[m]*KQ

            for _ in range(unroll):
                _emit_body(
                    nc, pmm, tpool, stage,
                    ztr, er, etr, mr, eor, ar, s_d,
                    m16, zT16, e16, eT16, zMT, u16, aT16, h8, etil8,
                    halfb, zerob, S, r,
                )

            if tiny_io:
                dstage = consts.tile([1, 1], F32)
                nc.any.memset(dstage, 1.0)
                nc.sync.dma_start(out=dout[:], in_=dstage[:])

    nc.compile()
    return nc


def _emit_body(nc, pmm, tpool, stage, ztr, er, etr, mr, eor, ar, s_d,
               m16, zT16, e16, eT16, zMT, u16, aT16, h8, etil8,
               halfb, zerob, S, r):
    # ---- loads (plain HWDGE, fp16 in DRAM), in consumption order ----
    for j in range(NT):
        nc.sync.dma_start(out=zT16[:, j, :], in_=ztr[j])
        nc.sync.dma_start(out=m16[:, j, :], in_=mr[j])
    for j in range(NT):
        nc.sync.dma_start(out=eT16[:, j, :], in_=etr[j])
    for j in range(NT):
        nc.sync.dma_start(out=e16[:, j, :], in_=er[j])

    # ---- mm1: zMT[e', n] = sum_d M[d, e'] * z[n, d] ----
    for h in range(2):
        for je in range(NT):
            ps = pmm.tile([P, 512], F32, tag="mm", bufs=4)
            for jd in range(NT):
                nc.tensor.matmul(
                    ps[:],
                    m16[:, jd, je * P:(je + 1) * P],
                    zT16[:, jd, h * 512:(h + 1) * 512],
                    start=(jd == 0),
                    stop=(jd == NT - 1),
                )
            nc.scalar.copy(out=zMT[:, je, h * 512:(h + 1) * 512], in_=ps[:])

    # ---- mm2 + fused softmax(sigmoid) per m-tile ----
    for jm in range(NT):
        for h in range(2):
            ps = pmm.tile([P, 512], F32, tag="mm", bufs=4)
            for je in range(NT):
                nc.tensor.matmul(
                    ps[:],
                    eT16[:, je, jm * P:(jm + 1) * P],
                    zMT[:, je, h * 512:(h + 1) * 512],
                    start=(je == 0),
                    stop=(je == NT - 1),
                )
            nc.scalar.activation(
                u16[:, jm, h * 512:(h + 1) * 512], ps[:], AF.Tanh,
                bias=zerob[:], scale=0.5,
            )
        t = tpool.tile([P, SZ], FP16, tag="t")
        nc.scalar.activation(
            t[:], u16[:, jm, :], AF.Exp,
            bias=halfb[:], scale=0.5,
            accum_out=S[:, jm:jm + 1],
        )
        nc.vector.reciprocal(r[:, jm:jm + 1], S[:, jm:jm + 1])
        nc.vector.tensor_scalar_mul(aT16[:, jm, :], t[:], r[:, jm:jm + 1])
        # A output: the fp16 softmax tile goes out directly (stored
        # transposed; host fixes layout and upcasts)
        nc.sync.dma_start(out=ar[jm], in_=aT16[:, jm, :])
        # fp8 operands for mm3: h = (t - c)/beta, etil = e * r * KQ
        nc.vector.tensor_scalar(
            h8[:, jm, :], t[:], C_AFF, 1.0 / B_AFF,
            op0=ALU.subtract, op1=ALU.mult,
        )
        nc.vector.tensor_scalar(
            etil8[:, jm, :], e16[:, jm, :], r[:, jm:jm + 1], KQ,
            op0=ALU.mult, op1=ALU.mult,
        )

    # S out (host folds the rank-1 c*colsum term during the gather)
    nc.sync.dma_start(out=s_d[:], in_=S[:])

    # ---- mm3 (fp8 DoubleRow): eoT[d, n] = beta/KQ * sum_m etil8*h8 ----
    for jd in range(NT):
        for hn in range(2):
            ps = pmm.tile([P, 512], F32, tag="mm", bufs=4)
            for jk in range(4):
                nc.tensor.matmul(
                    ps[:],
                    etil8[:, 2 * jk:2 * jk + 2, jd * P:(jd + 1) * P],
                    h8[:, 2 * jk:2 * jk + 2, hn * 512:(hn + 1) * 512],
                    start=(jk == 0),
                    stop=(jk == 3),
                    perf_mode=DR,
                )
            st = stage.tile([P, 512], FP16, tag="eost")
            nc.vector.tensor_scalar_mul(st[:], ps[:], B_AFF / KQ)
            nc.sync.dma_start(out=eor[jd, :, hn * 512:(hn + 1) * 512], in_=st[:])


_NC_CACHE = None


def _get_nc():
    global _NC_CACHE
    if _NC_CACHE is None:
        _NC_CACHE = _build_nc()
    return _NC_CACHE


def kernel(z: np.ndarray, e: np.ndarray, M: np.ndarray):
    z = np.ascontiguousarray(np.asarray(z, dtype=np.float32))
    e = np.ascontiguousarray(np.asarray(e, dtype=np.float32))
    M = np.ascontiguousarray(np.asarray(M, dtype=np.float32))
    assert z.shape == (NC, SZ, SZ) and e.shape == (NC, SZ, SZ) and M.shape == (SZ, SZ)

    # host-side shard layout: fp16 shards, z and e also transposed.
    z16 = z.astype(np.float16)
    e16h = e.astype(np.float16)
    M16 = M.astype(np.float16)
    zT = np.ascontiguousarray(z16.transpose(0, 2, 1))
    eT = np.ascontiguousarray(e16h.transpose(0, 2, 1))

    nc = _get_nc()
    in_maps = [{"zT": zT[i], "e": e16h[i], "eT": eT[i], "M": M16}
               for i in range(NC)]
    res = run_bass_kernel_spmd(nc, in_maps, core_ids=list(range(NC))).results

    # device stores A and eo transposed ([m,n] / [d,n]); undo in the gather.
    A = np.stack([res[i]["A"] for i in range(NC)]).astype(np.float32)
    A = np.ascontiguousarray(A.transpose(0, 2, 1))
    eo = np.stack([res[i]["eoT"] for i in range(NC)]).astype(np.float32)
    eo = eo.transpose(0, 2, 1)
    # rank-1 c*colsum term: eo[n,d] += c * sum_m e[m,d]/S[m]  (exact, fp32)
    for i in range(NC):
        S_flat = res[i]["S"].astype(np.float64).T.reshape(-1)   # S[m], m=jm*128+p
        colsum = (1.0 / S_flat) @ e[i].astype(np.float64)
        eo[i] += (C_AFF * colsum)[None, :].astype(np.float32)
    return np.ascontiguousarray(eo), A


# revision 4
# speedup vs baseline: 2.5157x; 2.5157x over previous
"""Trainium2 Bass kernel for nn_Attention_4243427688485.

Computation (per batch b):
    a   = z_b @ M @ e_b^T            [N, ME]
    A   = softmax(sigmoid(a), dim=N) (softmax over the query axis N)
    eo  = A @ e_b                    [N, D]
Returns (eo, A) stacked over the batch.

Sharding: data-parallel over batch B=8 across the 8 NeuronCores (one batch
per core, M replicated).  No collectives.  Host uploads fp16 shards
(z/e pre-transposed); outputs come back fp16 and transposed where noted.

Per-core device program:
  - mm1 (fp16): zMT[e',n] = sum_d M[d,e'] z[n,d]
  - mm2 (fp16): aT[m,n]   = sum_e' e[m,e'] zM[n,e']; ScalarE evicts via
    tanh(a/2); softmax over n: t = exp(0.5u+0.5), accum_out row-sum S,
    DVE reciprocal r=1/S, aT16 = t*r (fp16) = the A output (transposed).
  - mm3 (fp8 DoubleRow, ~1.5-2x TensorE rate): exploits the sigmoid
    saturation structure.  t = exp(sigmoid(a)) clusters at exactly {1, e}
    (98.7% of entries saturate), so with the affine split
        t = c + beta*h,  c=(1+e)/2, beta=(e-1)/2,  h in {-1,+1} (mostly)
    h is EXACTLY representable in fp8e4 at the clusters.  Then
        eo[n,d] = c * colsum[d] + beta * sum_m h[m,n] * (e[m,d]/S[m])
    The beta-term runs as fp8e4 DoubleRow matmuls in the eoT orientation:
        eoT = (etil8)^T-style matmul: lhsT = etil8[m,d] = fp8(e*r*2048),
        rhs = h8[m,n], psum accumulates m in 4 double-row (256-wide) steps.
    The c*colsum term is rank-1 in n: the device outputs S ([128,8] fp32,
    4KB) and the host folds  eo += c * ((1/S) @ e)  exactly in fp32 during
    the gather (1M MACs/batch on host, negligible).
    fp8 quantization error on etil dominates: measured rel_err(eo) ~1.1e-2
    vs the 2e-2 gate (A output unchanged at ~2.8e-3).
"""

import numpy as np

import concourse.bass as bass
import concourse.mybir as mybir
import concourse.tile as tile
from concourse import bacc
from concourse.bass_utils import run_bass_kernel_spmd

AF = mybir.ActivationFunctionType
ALU = mybir.AluOpType
DR = mybir.MatmulPerfMode.DoubleRow
F32 = mybir.dt.float32
FP16 = mybir.dt.float16
FP8 = mybir.dt.float8e4

P = 128
NT = 8
SZ = 1024
NC = 8

C_AFF = (1.0 + float(np.e)) / 2.0     # 1.8591409142295225
B_AFF = (float(np.e) - 1.0) / 2.0     # 0.8591409142295225
KQ = 2048.0                           # etil prescale so fp8e4 sees ~unit values


def _build_nc(unroll: int = 1, tiny_io: bool = False) -> bass.Bass:
    nc = bacc.Bacc()

    if tiny_io:
        nc.declare_dram_parameter("tin", [1, 1], F32, isOutput=False)
        dout = nc.declare_dram_parameter("tout", [1, 1], F32, isOutput=True)
        zt_d = nc.dram_tensor("zti", [SZ, SZ], FP16)
        e_d = nc.dram_tensor("ei", [SZ, SZ], FP16)
        et_d = nc.dram_tensor("eti", [SZ, SZ], FP16)
        m_d = nc.dram_tensor("Mi", [SZ, SZ], FP16)
        eo_d = nc.dram_tensor("eoi", [SZ, SZ], FP16)
        a_d = nc.dram_tensor("Ai", [SZ, SZ], FP16)
        s_d = nc.dram_tensor("Si", [P, NT], F32)
    else:
        zt_d = nc.declare_dram_parameter("zT", [SZ, SZ], FP16, isOutput=False)
        e_d = nc.declare_dram_parameter("e", [SZ, SZ], FP16, isOutput=False)
        et_d = nc.declare_dram_parameter("eT", [SZ, SZ], FP16, isOutput=False)
        m_d = nc.declare_dram_parameter("M", [SZ, SZ], FP16, isOutput=False)
        eo_d = nc.declare_dram_parameter("eoT", [SZ, SZ], FP16, isOutput=True)
        a_d = nc.declare_dram_parameter("A", [SZ, SZ], FP16, isOutput=True)
        s_d = nc.declare_dram_parameter("S", [P, NT], F32, isOutput=True)

    ztr = zt_d.rearrange("(j p) d -> j p d", p=P)
    er = e_d.rearrange("(j p) d -> j p d", p=P)
    etr = et_d.rearrange("(j p) d -> j p d", p=P)
    mr = m_d.rearrange("(j p) d -> j p d", p=P)
    eor = eo_d.rearrange("(j p) d -> j p d", p=P)
    ar = a_d.rearrange("(j p) d -> j p d", p=P)

    with tile.TileContext(nc) as tc:
        with (
            tc.tile_pool(name="big", bufs=1) as big,
            tc.tile_pool(name="consts", bufs=1) as consts,
            tc.tile_pool(name="tpool", bufs=4) as tpool,
            tc.tile_pool(name="stage", bufs=8) as stage,
            tc.tile_pool(name="psum_mm", bufs=1, space="PSUM") as pmm,
        ):
            halfb = consts.tile([P, 1], F32)
            nc.any.memset(halfb, 0.5)
            zerob = consts.tile([P, 1], F32)
            nc.any.memset(zerob, 0.0)
            S = consts.tile([P, NT], F32)
            r = consts.tile([P, NT], F32)

            m16 = big.tile([P, NT, SZ], FP16)    # m16[p, jd, e'] = M[jd*128+p, e']
            zT16 = big.tile([P, NT, SZ], FP16)   # zT16[p, jd, n] = z[n, jd*128+p]
            e16 = big.tile([P, NT, SZ], FP16)    # e16[p, jm, d]  = e[jm*128+p, d]
            eT16 = big.tile([P, NT, SZ], FP16)   # eT16[p, je, m] = e[m, je*128+p]
            zMT = big.tile([P, NT, SZ], FP16)    # zMT[p, je, n]  = (z@M)[n, je*128+p]
            u16 = big.tile([P, NT, SZ], FP16)    # u[p, jm, n]    = tanh(a[n, jm*128+p]/2)
            aT16 = big.tile([P, NT, SZ], FP16)   # aT16[p, jm, n] = A[n, jm*128+p]
            h8 = big.tile([P, NT, SZ], FP8)      # h8[p, jm, n]   = (t - c)/beta
            etil8 = big.tile([P, NT, SZ], FP8)   # etil8[p, jm, d]= e[m,d]*r[m]*KQ

            for _ in range(unroll):
                _emit_body(
                    nc, pmm, tpool, stage,
                    ztr, er, etr, mr, eor, ar, s_d,
                    m16, zT16, e16, eT16, zMT, u16, aT16, h8, etil8,
                    halfb, zerob, S, r,
                )

            if tiny_io:
                dstage = consts.tile([1, 1], F32)
                nc.any.memset(dstage, 1.0)
                nc.sync.dma_start(out=dout[:], in_=dstage[:])

    nc.compile()
    return nc


def _emit_body(nc, pmm, tpool, stage, ztr, er, etr, mr, eor, ar, s_d,
               m16, zT16, e16, eT16, zMT, u16, aT16, h8, etil8,
               halfb, zerob, S, r):
    # ---- loads (plain HWDGE, fp16 in DRAM), in consumption order ----
    for j in range(NT):
        nc.sync.dma_start(out=zT16[:, j, :], in_=ztr[j])
        nc.sync.dma_start(out=m16[:, j, :], in_=mr[j])
    for j in range(NT):
        nc.sync.dma_start(out=eT16[:, j, :], in_=etr[j])
    for j in range(NT):
        nc.sync.dma_start(out=e16[:, j, :], in_=er[j])

    # ---- mm1: zMT[e', n] = sum_d M[d, e'] * z[n, d] ----
    for h in range(2):
        for je in range(NT):
            ps = pmm.tile([P, 512], F32, tag="mm", bufs=4)
            for jd in range(NT):
                nc.tensor.matmul(
                    ps[:],
                    m16[:, jd, je * P:(je + 1) * P],
                    zT16[:, jd, h * 512:(h + 1) * 512],
                    start=(jd == 0),
                    stop=(jd == NT - 1),
                )
            nc.scalar.copy(out=zMT[:, je, h * 512:(h + 1) * 512], in_=ps[:])

    # ---- mm2 + fused softmax(sigmoid) per m-tile ----
    for jm in range(NT):
        for h in range(2):
            ps = pmm.tile([P, 512], F32, tag="mm", bufs=4)
            for je in range(NT):
                nc.tensor.matmul(
                    ps[:],
                    eT16[:, je, jm * P:(jm + 1) * P],
                    zMT[:, je, h * 512:(h + 1) * 512],
                    start=(je == 0),
                    stop=(je == NT - 1),
                )
            nc.scalar.activation(
                u16[:, jm, h * 512:(h + 1) * 512], ps[:], AF.Tanh,
                bias=zerob[:], scale=0.5,
            )
        t = tpool.tile([P, SZ], FP16, tag="t")
        nc.scalar.activation(
            t[:], u16[:, jm, :], AF.Exp,
            bias=halfb[:], scale=0.5,
            accum_out=S[:, jm:jm + 1],
        )
        nc.vector.reciprocal(r[:, jm:jm + 1], S[:, jm:jm + 1])
        # fp8 operands for mm3 first (they gate mm3's start; aT16 doesn't):
        # h = (t - c)/beta, etil = e * r * KQ
        nc.vector.tensor_scalar(
            h8[:, jm, :], t[:], C_AFF, 1.0 / B_AFF,
            op0=ALU.subtract, op1=ALU.mult,
        )
        nc.vector.tensor_scalar(
            etil8[:, jm, :], e16[:, jm, :], r[:, jm:jm + 1], KQ,
            op0=ALU.mult, op1=ALU.mult,
        )
        nc.vector.tensor_scalar_mul(aT16[:, jm, :], t[:], r[:, jm:jm + 1])
        # A output: the fp16 softmax tile goes out directly (stored
        # transposed; host fixes layout and upcasts)
        nc.sync.dma_start(out=ar[jm], in_=aT16[:, jm, :])

    # S out (host folds the rank-1 c*colsum term during the gather)
    nc.sync.dma_start(out=s_d[:], in_=S[:])

    # ---- mm3 (fp8 DoubleRow): eoT[d, n] = beta/KQ * sum_m etil8*h8 ----
    # The first four psum tiles are staged (jk=0..2 for all four, then the
    # jk=3 round) so the jk=3 operands — h8/etil8 of the last two m-tiles,
    # which are only ready ~2us after mm2's last matmul — are not needed
    # until ~4 tiles worth of DR matmuls have been issued.
    def dr_mm(ps, jd, hn, jk):
        nc.tensor.matmul(
            ps[:],
            etil8[:, 2 * jk:2 * jk + 2, jd * P:(jd + 1) * P],
            h8[:, 2 * jk:2 * jk + 2, hn * 512:(hn + 1) * 512],
            start=(jk == 0),
            stop=(jk == 3),
            perf_mode=DR,
        )

    def dr_evict(ps, jd, hn):
        st = stage.tile([P, 512], FP16, tag="eost")
        nc.scalar.activation(st[:], ps[:], AF.Copy, bias=zerob[:], scale=B_AFF / KQ)
        nc.sync.dma_start(out=eor[jd, :, hn * 512:(hn + 1) * 512], in_=st[:])

    head = [(0, 0), (0, 1), (1, 0), (1, 1)]
    head_ps = {}
    for (jd, hn) in head:
        head_ps[(jd, hn)] = pmm.tile([P, 512], F32, tag="mm", bufs=4)
        for jk in range(3):
            dr_mm(head_ps[(jd, hn)], jd, hn, jk)
    for (jd, hn) in head:
        dr_mm(head_ps[(jd, hn)], jd, hn, 3)
        dr_evict(head_ps[(jd, hn)], jd, hn)
    for jd in range(2, NT):
        for hn in range(2):
            ps = pmm.tile([P, 512], F32, tag="mm", bufs=4)
            for jk in range(4):
                dr_mm(ps, jd, hn, jk)
            dr_evict(ps, jd, hn)


_NC_CACHE = None


def _get_nc():
    global _NC_CACHE
    if _NC_CACHE is None:
        _NC_CACHE = _build_nc()
    return _NC_CACHE


def kernel(z: np.ndarray, e: np.ndarray, M: np.ndarray):
    z = np.ascontiguousarray(np.asarray(z, dtype=np.float32))
    e = np.ascontiguousarray(np.asarray(e, dtype=np.float32))
    M = np.ascontiguousarray(np.asarray(M, dtype=np.float32))
    assert z.shape == (NC, SZ, SZ) and e.shape == (NC, SZ, SZ) and M.shape == (SZ, SZ)

    # host-side shard layout: fp16 shards, z and e also transposed.
    z16 = z.astype(np.float16)
    e16h = e.astype(np.float16)
    M16 = M.astype(np.float16)
    zT = np.ascontiguousarray(z16.transpose(0, 2, 1))
    eT = np.ascontiguousarray(e16h.transpose(0, 2, 1))

    nc = _get_nc()
    in_maps = [{"zT": zT[i], "e": e16h[i], "eT": eT[i], "M": M16}
               for i in range(NC)]
    res = run_bass_kernel_spmd(nc, in_maps, core_ids=list(range(NC))).results

    # device stores A and eo transposed ([m,n] / [d,n]); undo in the gather.
    A = np.stack([res[i]["A"] for i in range(NC)]).astype(np.float32)
    A = np.ascontiguousarray(A.transpose(0, 2, 1))
    eo = np.stack([res[i]["eoT"] for i in range(NC)]).astype(np.float32)
    eo = eo.transpose(0, 2, 1)
    # rank-1 c*colsum term: eo[n,d] += c * sum_m e[m,d]/S[m]  (exact, fp32)
    for i in range(NC):
        S_flat = res[i]["S"].astype(np.float64).T.reshape(-1)   # S[m], m=jm*128+p
        colsum = (1.0 / S_flat) @ e[i].astype(np.float64)
        eo[i] += (C_AFF * colsum)[None, :].astype(np.float32)
    return np.ascontiguousarray(eo), A
